# revision 1
# baseline (speedup 1.0000x reference)
"""DCN block kernel for Trainium2 (8 NeuronCores, data-parallel over batch).

Math (per batch b, plane c):
  z   = conv3x3(x, w_off) + b_off                  (64 offset logits)
  d   = sigmoid(z) - 0.5   in (-.5, .5)            (pixel displacement)
  sample at (r - dy, c - dx) bilinear w/ reflect   (|d| < .5 => 3x3 support!)
  y   = conv3x3(sampled, w_dcn) + b_dcn

Because |d| < 0.5 the bilinear gather only touches the 3x3 neighborhood, so it
is computed gather-free as
  H(sigma)  = x + dxt*AR + |dxt|*BR     (AR = x(c-1)-x(c+1), BR = x(c-1)+x(c+1)-2x)
  out = H0 + dyt*(Hm-Hp) + |dyt|*(Hm+Hp-2H0)
with dxt = d/2. With reflect-consistent fixups at image border rows/cols this
is exact.

Layout: 4 image row-quarters stacked on partition groups [4 x 32ch]; convs run
as 4 concurrent row-tiled matmul streams (tile_position), K=32, 9 taps
accumulating in PSUM; elementwise sampling runs on [128, fd] bf16 tiles.
"""

import math
from contextlib import ExitStack

import ml_dtypes
import numpy as np

import concourse.bacc as bacc
import concourse.bass as bass
import concourse.mybir as mybir
import concourse.tile as tile

BF16 = mybir.dt.bfloat16
F32 = mybir.dt.float32
AF = mybir.ActivationFunctionType
OP = mybir.AluOpType

N_CORES = 8
C = 32          # input/output channels per plane set
OC2 = 64        # offset logits (2 per plane)


class Cfg:
    def __init__(self, H=384, nr=8):
        self.H = H
        self.W = H
        self.WP = self.W + 2          # padded row: [pad, 0..W-1, pad]
        self.QH = H // 4              # rows per quarter
        assert self.QH % nr == 0
        self.nr = nr                  # output rows per quarter per slab
        self.nslab = self.QH // nr


def _f(ap):
    """Flatten free dims of a 3d tile AP to [P, fd]."""
    return ap.rearrange("p a b -> p (a b)")


def build_nc(cfg: Cfg, finalize=True):
    nc = bacc.Bacc()
    H, W, WP, nr = cfg.H, cfg.W, cfg.WP, cfg.nr

    x_in = nc.declare_dram_parameter("x", [C, H + 4, W], BF16, isOutput=False)
    woff_in = nc.declare_dram_parameter("woff", [128, 9 * OC2], BF16, isOutput=False)
    wdcn_in = nc.declare_dram_parameter("wdcn", [128, 9 * C], BF16, isOutput=False)
    boff_in = nc.declare_dram_parameter("boff", [128, 1], F32, isOutput=False)
    bdcn_in = nc.declare_dram_parameter("bdcn", [128, 1], F32, isOutput=False)
    y_out = nc.declare_dram_parameter("y", [C, H, W], F32, isOutput=True)

    with tile.TileContext(nc) as tc, ExitStack() as ctx:
        fold_sem = ctx.enter_context(nc.semaphore("fold_sem"))
        fold_cnt = [0]
        store_sem = ctx.enter_context(nc.semaphore("store_sem"))
        store_cnt = [0]
        consts = ctx.enter_context(tc.tile_pool(name="consts", bufs=1))
        xpool = ctx.enter_context(tc.tile_pool(name="xp", bufs=1))
        abpool = ctx.enter_context(tc.tile_pool(name="abp", bufs=1))
        spool = ctx.enter_context(tc.tile_pool(name="sp", bufs=1))
        mpool = ctx.enter_context(tc.tile_pool(name="mp", bufs=1))
        hpool = ctx.enter_context(tc.tile_pool(name="hp", bufs=1))
        ospool = ctx.enter_context(tc.tile_pool(name="osp", bufs=1))
        ocpool = ctx.enter_context(tc.tile_pool(name="ocp", bufs=2))
        zpool = ctx.enter_context(tc.tile_pool(name="zp", bufs=2, space="PSUM"))
        opool = ctx.enter_context(tc.tile_pool(name="op", bufs=2, space="PSUM"))

        WOFF = consts.tile([128, 9, OC2], BF16)
        nc.sync.dma_start(out=_f(WOFF), in_=woff_in[:])
        WDCN = consts.tile([128, 9, C], BF16)
        nc.sync.dma_start(out=_f(WDCN), in_=wdcn_in[:])
        BOFF = consts.tile([128, 1], F32)
        nc.sync.dma_start(out=BOFF[:], in_=boff_in[:])
        BDCN = consts.tile([128, 1], F32)
        nc.sync.dma_start(out=BDCN[:], in_=bdcn_in[:])
        WU = consts.tile([128, 1], F32)
        nc.scalar.activation(out=WU[:], in_=BOFF[:], func=AF.Identity,
                             bias=BDCN[:], scale=1.0)
        NEG25 = consts.tile([128, 1], F32)
        nc.vector.memset(NEG25[:], -0.25)

        nh = nr + 2   # z/s/H/OS rows: [r0-1, r0+nr+1)
        nx = nr + 4   # x rows:       [r0-2, r0+nr+2)
        # persistent x slabs (ping-pong): pre-zero once -> pad cols and
        # first-slab halo rows stay zero forever
        XS_pp = []
        for sl in range(2):
            XSz = xpool.tile([128, nx, WP], BF16, tag=f"xs{sl}", name=f"xsz{sl}")
            nc.vector.memset(_f(XSz), 0.0)
            XS_pp.append(XSz)
        # persistent sigmoid-pair tiles: pad cols pre-zeroed once
        SP = []
        for p in range(2):
            SPp = spool.tile([128, nh, WP], BF16, tag=f"sp{p}", name=f"spp{p}")
            nc.vector.memset(SPp[:, :, 0:WP:W + 1], 0.0)
            SP.append(SPp)

        for it in range(cfg.nslab):
            r0 = it * nr

            # ---- load x slab: 4 quarters stacked on partition groups ----
            XS = XS_pp[it % 2]
            for g in range(4):
                i0 = cfg.QH * g + r0   # row in the padded-x frame
                nc.sync.dma_start(
                    out=XS[32 * g:32 * g + 32, :, 1:W + 1],
                    in_=x_in[:, i0:i0 + nx, :])
            XSf = _f(XS[:])

            # ---- conv_off + sigmoid, pairs (q0,q1)->ztA, (q2,q3)->ztB ----
            for zi in range(nh):
                zts = [zpool.tile([128, 512], F32, tag=f"z{p}", name=f"zt{p}")
                       for p in range(2)]
                for t in range(9):
                    kh, kw = t // 3, t % 3
                    base = (zi + kh) * WP + kw
                    for p in range(2):
                        for gq in range(2):
                            g = 2 * p + gq
                            nc.tensor.matmul(
                                zts[p][64 * gq:64 * gq + 64, 1:W + 1],
                                lhsT=WOFF[32 * g:32 * g + 32, t, :],
                                rhs=XSf[32 * g:32 * g + 32, base:base + W],
                                start=(t == 0), stop=(t == 8),
                                tile_position=(32 * g, 64 * gq),
                                skip_group_check=True)
                for p in range(2):
                    nc.scalar.activation(
                        out=SP[p][:, zi, 1:W + 1], in_=zts[p][:, 1:W + 1],
                        func=AF.Sigmoid, bias=BOFF[:], scale=1.0)

            # ---- fold s into quarter-stacked SX/SY (8 sbuf->sbuf DMAs) ----
            SX = spool.tile([128, nh, WP], BF16, tag="sx")
            SY = spool.tile([128, nh, WP], BF16, tag="sy")
            with tc.tile_critical():
                for g in range(4):
                    p, gq = g // 2, g % 2
                    nc.gpsimd.dma_start(
                        out=_f(SX[32 * g:32 * g + 32]),
                        in_=_f(SP[p][64 * gq:64 * gq + 32])).then_inc(fold_sem, 16)
                    nc.gpsimd.dma_start(
                        out=_f(SY[32 * g:32 * g + 32]),
                        in_=_f(SP[p][64 * gq + 32:64 * gq + 64])).then_inc(fold_sem, 16)
                fold_cnt[0] += 128
                nc.gpsimd.wait_ge(fold_sem, fold_cnt[0])

            # ---- displacement maps: dxt = s/2 - 1/4, |dxt| ----
            DX = mpool.tile([128, nh, WP], BF16, tag="dx")
            nc.vector.tensor_scalar(_f(DX), _f(SX), 0.5, -0.25, OP.mult, OP.add)
            ADX = mpool.tile([128, nh, WP], BF16, tag="adx")
            nc.scalar.activation(out=_f(ADX), in_=_f(SX), func=AF.Abs,
                                 bias=NEG25[:], scale=0.5)

            # ---- column diff images on x geometry ----
            Lx = nx * WP
            AR = abpool.tile([128, nx, WP], BF16, tag="ar")
            ARf = _f(AR)
            nc.vector.tensor_tensor(
                ARf[:, 1:Lx - 1], XSf[:, 0:Lx - 2], XSf[:, 2:Lx], OP.subtract)
            BR0 = abpool.tile([128, nx, WP], BF16, tag="br0")
            BR0f = _f(BR0)
            nc.gpsimd.tensor_tensor(
                BR0f[:, 1:Lx - 1], XSf[:, 0:Lx - 2], XSf[:, 2:Lx], OP.add)
            BR = abpool.tile([128, nx, WP], BF16, tag="br")
            BRf = _f(BR)
            nc.vector.scalar_tensor_tensor(
                BRf[:, 1:Lx - 1], in0=XSf[:, 1:Lx - 1], scalar=-2.0,
                in1=BR0f[:, 1:Lx - 1], op0=OP.mult, op1=OP.add)
            for tl in (ARf, BRf):
                nc.vector.memset(tl[:, 0:1], 0.0)
                nc.vector.memset(tl[:, Lx - 1:Lx], 0.0)
            # reflect fixups at image cols 0 / W-1 (padded cols 1 / W)
            nc.vector.memset(AR[:, :, 1], 0.0)
            nc.vector.memset(AR[:, :, W], 0.0)
            nc.vector.tensor_tensor(
                BR[:, :, 1], BR[:, :, 1], XS[:, :, 2], OP.add)
            nc.vector.tensor_tensor(
                BR[:, :, W], BR[:, :, W], XS[:, :, W - 1], OP.add)

            # ---- horizontal interps H(-1), H(0), H(+1) ----
            Lh = nh * WP
            DXf, ADXf = _f(DX), _f(ADX)
            Hs = []
            for dr in (-1, 0, 1):
                off = (1 + dr) * WP
                T1 = hpool.tile([128, nh, WP], BF16, tag="ht1")
                nc.vector.tensor_tensor(
                    _f(T1), DXf, ARf[:, off:off + Lh], OP.mult)
                T2 = hpool.tile([128, nh, WP], BF16, tag="ht2")
                nc.vector.tensor_tensor(
                    _f(T2), ADXf, BRf[:, off:off + Lh], OP.mult)
                Hd = hpool.tile([128, nh, WP], BF16, tag=f"h{dr}")
                nc.vector.tensor_tensor(
                    _f(Hd), _f(T1), XSf[:, off:off + Lh], OP.add)
                nc.vector.tensor_tensor(_f(Hd), _f(Hd), _f(T2), OP.add)
                Hs.append(Hd)
            Hm, H0, Hp = Hs

            # ---- vertical combine ----
            AH = hpool.tile([128, nh, WP], BF16, tag="ah")
            nc.vector.tensor_tensor(_f(AH), _f(Hm), _f(Hp), OP.subtract)
            BH0 = hpool.tile([128, nh, WP], BF16, tag="ht2")
            nc.vector.tensor_tensor(_f(BH0), _f(Hm), _f(Hp), OP.add)
            BH = hpool.tile([128, nh, WP], BF16, tag="bh")
            nc.vector.scalar_tensor_tensor(
                _f(BH), in0=_f(H0), scalar=-2.0, in1=_f(BH0),
                op0=OP.mult, op1=OP.add)
            # y displacement maps (reuse the dx/adx slots)
            DY = mpool.tile([128, nh, WP], BF16, tag="dx")
            nc.vector.tensor_scalar(_f(DY), _f(SY), 0.5, -0.25, OP.mult, OP.add)
            ADY = mpool.tile([128, nh, WP], BF16, tag="adx")
            nc.scalar.activation(out=_f(ADY), in_=_f(SY), func=AF.Abs,
                                 bias=NEG25[:], scale=0.5)
            # reflect fixups at image rows 0 / H-1 (Hm/Hp read zero rows there)
            if it == 0:
                nc.vector.memset(_f(AH[0:32, 1:2, :]), 0.0)
                nc.vector.tensor_tensor(
                    _f(BH[0:32, 1:2, :]), _f(BH[0:32, 1:2, :]),
                    _f(Hp[0:32, 1:2, :]), OP.add)
            if it == cfg.nslab - 1:
                nc.vector.memset(_f(AH[96:128, nr:nr + 1, :]), 0.0)
                nc.vector.tensor_tensor(
                    _f(BH[96:128, nr:nr + 1, :]), _f(BH[96:128, nr:nr + 1, :]),
                    _f(Hm[96:128, nr:nr + 1, :]), OP.add)

            OS = ospool.tile([128, nh, WP], BF16)
            T3 = hpool.tile([128, nh, WP], BF16, tag="ht1")
            nc.vector.tensor_tensor(_f(T3), _f(DY), _f(AH), OP.mult)
            T4 = hpool.tile([128, nh, WP], BF16, tag="ht2")
            nc.vector.tensor_tensor(_f(T4), _f(ADY), _f(BH), OP.mult)
            nc.vector.tensor_tensor(_f(OS), _f(H0), _f(T3), OP.add)
            nc.vector.tensor_tensor(_f(OS), _f(OS), _f(T4), OP.add)
            # sampled outside the image is 0 for the final conv zero-padding
            nc.vector.memset(OS[:, :, 0:WP:W + 1], 0.0)
            if it == 0:
                nc.vector.memset(_f(OS[0:32, 0:1, :]), 0.0)
            if it == cfg.nslab - 1:
                nc.vector.memset(_f(OS[96:128, nr + 1:nr + 2, :]), 0.0)
            OSf = _f(OS)

            # ---- conv_dcn + bias + store (4-row output chunks) ----
            OROWS = 4
            for oc_i in range(nr // OROWS):
                OC = ocpool.tile([128, OROWS, WP], F32)
                for oj in range(OROWS):
                    oi = oc_i * OROWS + oj
                    ot = opool.tile([128, 512], F32)
                    for t in range(9):
                        kh, kw = t // 3, t % 3
                        base = (oi + kh) * WP + kw
                        for g in range(4):
                            nc.tensor.matmul(
                                ot[32 * g:32 * g + 32, 1:W + 1],
                                lhsT=WDCN[32 * g:32 * g + 32, t, :],
                                rhs=OSf[32 * g:32 * g + 32, base:base + W],
                                start=(t == 0), stop=(t == 8),
                                tile_position=(32 * g, 32 * g),
                                skip_group_check=True)
                    nc.scalar.activation(
                        out=OC[:, oj, 1:W + 1], in_=ot[:, 1:W + 1],
                        func=AF.Identity, bias=BDCN[:], scale=1.0)
                with tc.tile_critical():
                    for g in range(4):
                        rr = cfg.QH * g + r0 + oc_i * OROWS
                        nc.gpsimd.dma_start(
                            out=y_out[:, rr:rr + OROWS, :],
                            in_=OC[32 * g:32 * g + 32, :, 1:W + 1]
                        ).then_inc(store_sem, 16)
                    store_cnt[0] += 64
                    nc.gpsimd.wait_ge(store_sem, store_cnt[0])
    if finalize:
        nc.finalize()
    return nc


def prep_weights(w_off, b_off, w_dcn, b_dcn):
    """Host-side packing of conv weights into lhsT tiles, replicated x4."""
    perm = np.concatenate([np.arange(0, 2 * C, 2), np.arange(1, 2 * C, 2)])
    # WOFF[32g+ci, kh*3+kw, m] = w_off[perm[m], ci, kh, kw]
    wo = w_off[perm].astype(np.float32)            # [64, C, 3, 3]
    wo = wo.transpose(1, 2, 3, 0).reshape(C, 9, OC2)   # [ci, tap, m]
    woff = np.tile(wo, (4, 1, 1)).reshape(128, 9 * OC2)
    wd = w_dcn.astype(np.float32).transpose(1, 2, 3, 0).reshape(C, 9, C)
    wdcn = np.tile(wd, (4, 1, 1)).reshape(128, 9 * C)
    boff = np.tile(b_off[perm].astype(np.float32), 2).reshape(128, 1)
    bdcn = np.tile(b_dcn.astype(np.float32), 4).reshape(128, 1)
    return {
        "woff": woff.astype(ml_dtypes.bfloat16),
        "wdcn": wdcn.astype(ml_dtypes.bfloat16),
        "boff": boff.astype(np.float32),
        "bdcn": bdcn.astype(np.float32),
    }


_NC_CACHE = {}


def _get_nc(cfg_key):
    if cfg_key not in _NC_CACHE:
        _NC_CACHE[cfg_key] = build_nc(Cfg(H=cfg_key[0], nr=cfg_key[1]))
    return _NC_CACHE[cfg_key]


def _run(x, w_off, b_off, w_dcn, b_dcn, **spmd_kwargs):
    from concourse.bass_utils import run_bass_kernel_spmd

    B = x.shape[0]
    H = x.shape[2]
    assert x.shape == (B, C, H, H) and B == N_CORES
    nc = _get_nc((H, 8))
    w = prep_weights(np.asarray(w_off), np.asarray(b_off),
                     np.asarray(w_dcn), np.asarray(b_dcn))
    in_maps = []
    for b in range(B):
        m = dict(w)
        xb = np.asarray(x[b]).astype(ml_dtypes.bfloat16)
        m["x"] = np.pad(xb, ((0, 0), (2, 2), (0, 0)))
        in_maps.append(m)
    return run_bass_kernel_spmd(nc, in_maps, list(range(N_CORES)), **spmd_kwargs)


def kernel(x, w_off, b_off, w_dcn, b_dcn):
    res = _run(x, w_off, b_off, w_dcn, b_dcn)
    out = np.stack([res.results[i]["y"] for i in range(N_CORES)], axis=0)
    return out.astype(np.float32)



# revision 2
# speedup vs baseline: 1.0325x; 1.0325x over previous
"""DCN block kernel v2 for Trainium2 (8 cores, data-parallel over batch).

Per core (one batch image, C=32 planes, 384x384):
  z = conv3x3(x, w_off)+b_off; s = sigmoid(z); d = s-.5 in (-.5,.5)
  sample at (r-dy, c-dx) bilinear w/ reflect  (|d|<.5 -> 3x3 support)
  y = conv3x3(sampled, w_dcn)+b_dcn

v2 structure:
  conv_off: K=128 M=128 "pair" matmuls; rhs = stride-2 row-interleaved x
    (host-prepped DRAM). One matmul per kw tap -> TWO sigmoid rows x 64
    maps. 192 pairs x 3 taps, no halo recompute.
  sigmoid: ACT PSUM->SBUF pair tiles [128 = 2rows x (32dx|32dy), W].
  fold: pair tiles -> parity-split quarter-stack planes SXe/SXo/SYe/SYo
    (contiguous multi-KB DMA runs).
  sampling (|d|-form, quarter-stack [32u+c, rows, W]):
    H(dr) = X + dxt*AR + |dxt|*BR, dr in {-1,0,1}; vertical combine.
    ACT: ADX/ADY/2H0; GpSimd: BR0 + one H-add; DVE: the rest.
    Final adds write parity planes OSe/OSo (ring of 3 slabs).
  conv_dcn (lagged 1 slab): OSR stride-2 fold from OS parity planes,
    K=128 M=64 col-paired matmuls (pairs t,t+1 concurrent on PE column
    halves). Quarter-boundary pairs deferred to a cleanup pass using
    stashed first-slab rows. ACT bias-copy -> bf16 -> DRAM; host
    transposes/casts.
"""

from contextlib import ExitStack

import ml_dtypes
import numpy as np

import concourse.bacc as bacc
import concourse.bass as bass
import concourse.mybir as mybir
import concourse.tile as tile

BF16 = mybir.dt.bfloat16
F32 = mybir.dt.float32
AF = mybir.ActivationFunctionType
OP = mybir.AluOpType

N_CORES = 8
C = 32
H = 384
W = 384
WP = W + 2
QH = H // 4       # 96 rows per quarter
NR = 8            # image rows per quarter per slab
NSLAB = QH // NR  # 12
NPAIR = NR // 2   # 4 row-pairs per band-slab


def build_nc(finalize=True):
    nc = bacc.Bacc()
    xr_in = nc.declare_dram_parameter("xr", [4, C, H // 2, WP], BF16, isOutput=False)
    xq_in = nc.declare_dram_parameter("xq", [4, C, QH + 4, WP], BF16, isOutput=False)
    woff_in = nc.declare_dram_parameter("woff", [128, 3 * 128], BF16, isOutput=False)
    wdcn_in = nc.declare_dram_parameter("wdcn", [128, 3 * 64], BF16, isOutput=False)
    boff_in = nc.declare_dram_parameter("boff", [128, 1], F32, isOutput=False)
    bdcn_in = nc.declare_dram_parameter("bdcn", [128, 1], F32, isOutput=False)
    # y[t, h, o, w] = out(plane o, row 2t+h, col w)
    y_out = nc.declare_dram_parameter("y", [H // 2, 2, C, W], BF16, isOutput=True)

    with tile.TileContext(nc) as tc, ExitStack() as ctx:
        consts = ctx.enter_context(tc.tile_pool(name="consts", bufs=1))
        xrpool = ctx.enter_context(tc.tile_pool(name="xrp", bufs=2))
        xspool = ctx.enter_context(tc.tile_pool(name="xsp", bufs=2))
        spool = ctx.enter_context(tc.tile_pool(name="sp", bufs=1))
        sxpool = ctx.enter_context(tc.tile_pool(name="sxp", bufs=1))
        mpool = ctx.enter_context(tc.tile_pool(name="mp", bufs=1))
        abpool = ctx.enter_context(tc.tile_pool(name="abp", bufs=1))
        hpool = ctx.enter_context(tc.tile_pool(name="hp", bufs=1))
        ospool = ctx.enter_context(tc.tile_pool(name="osp", bufs=1))
        osrpool = ctx.enter_context(tc.tile_pool(name="osrp", bufs=1))
        stpool = ctx.enter_context(tc.tile_pool(name="stp", bufs=2))
        zpool = ctx.enter_context(tc.tile_pool(name="zp", bufs=3, space="PSUM"))
        opool = ctx.enter_context(tc.tile_pool(name="op", bufs=3, space="PSUM"))

        WOFF = consts.tile([128, 3, 128], BF16)
        nc.sync.dma_start(out=WOFF[:].rearrange("p a b -> p (a b)"), in_=woff_in[:])
        WDCN = consts.tile([128, 3, 64], BF16)
        nc.sync.dma_start(out=WDCN[:].rearrange("p a b -> p (a b)"), in_=wdcn_in[:])
        BOFF = consts.tile([128, 1], F32)
        nc.sync.dma_start(out=BOFF[:], in_=boff_in[:])
        BDCN = consts.tile([128, 1], F32)
        nc.sync.dma_start(out=BDCN[:], in_=bdcn_in[:])
        NQ = consts.tile([128, 1], F32)
        nc.vector.memset(NQ[:], -0.25)
        ZB = consts.tile([128, 1], F32)
        nc.vector.memset(ZB[:], 0.0)

        # OS parity planes, ring of 3 slabs; pad cols pre-zeroed once.
        OSE, OSO = [], []
        for r in range(3):
            te = ospool.tile([128, NPAIR, WP], BF16, tag=f"ose{r}", name=f"ose{r}")
            to = ospool.tile([128, NPAIR, WP], BF16, tag=f"oso{r}", name=f"oso{r}")
            nc.vector.memset(te[:].rearrange("p a b -> p (a b)"), 0.0)
            nc.vector.memset(to[:].rearrange("p a b -> p (a b)"), 0.0)
            OSE.append(te)
            OSO.append(to)
        # stash of first-slab rows 96u+{0,1,2} for the boundary cleanup
        STE = consts.tile([128, 2, WP], BF16)
        STO = consts.tile([128, 1, WP], BF16)

        def sample_slab(it):
            r0 = it * NR
            b0 = r0 // 2
            # ---- conv_off: 4 bands x NPAIR pairs ----
            XR = xrpool.tile([128, 4, NPAIR, WP], BF16, tag="xr")
            for g in range(4):
                for u in range(4):
                    nc.sync.dma_start(
                        out=XR[32 * g:32 * g + 32, u],
                        in_=xr_in[g, :, 48 * u + b0:48 * u + b0 + NPAIR, :])
            SB = spool.tile([128, 4, NPAIR, W], BF16, tag="sb")
            for u in range(4):
                for k in range(NPAIR):
                    ps = zpool.tile([128, 512], F32, tag="z")
                    for kw in range(3):
                        nc.tensor.matmul(
                            ps[:, 0:W],
                            lhsT=WOFF[:, kw, :],
                            rhs=XR[:, u, k, kw:kw + W],
                            start=(kw == 0), stop=(kw == 2))
                    nc.scalar.activation(out=SB[:, u, k, :], in_=ps[:, 0:W],
                                         func=AF.Sigmoid, bias=BOFF[:], scale=1.0)
            # ---- fold to parity-split quarter-stack planes ----
            SXE = sxpool.tile([128, NPAIR, W], BF16, tag="sxe")
            SXO = sxpool.tile([128, NPAIR, W], BF16, tag="sxo")
            SYE = sxpool.tile([128, NPAIR, W], BF16, tag="sye")
            SYO = sxpool.tile([128, NPAIR, W], BF16, tag="syo")
            for u in range(4):
                nc.sync.dma_start(out=SXE[32 * u:32 * u + 32], in_=SB[0:32, u])
                nc.sync.dma_start(out=SYE[32 * u:32 * u + 32], in_=SB[32:64, u])
                nc.sync.dma_start(out=SXO[32 * u:32 * u + 32], in_=SB[64:96, u])
                nc.sync.dma_start(out=SYO[32 * u:32 * u + 32], in_=SB[96:128, u])
            # ---- displacement maps [128, NR, W] ----
            DX = mpool.tile([128, NR, W], BF16, tag="dx")
            ADX = mpool.tile([128, NR, W], BF16, tag="adx")
            DY = mpool.tile([128, NR, W], BF16, tag="dy")
            ADY = mpool.tile([128, NR, W], BF16, tag="ady")
            for (pe, po, d, ad) in ((SXE, SXO, DX, ADX), (SYE, SYO, DY, ADY)):
                nc.vector.tensor_scalar(d[:, 0:NR:2, :], pe[:], 0.5, -0.25,
                                        OP.mult, OP.add)
                nc.vector.tensor_scalar(d[:, 1:NR:2, :], po[:], 0.5, -0.25,
                                        OP.mult, OP.add)
                nc.scalar.activation(out=ad[:, 0:NR:2, :], in_=pe[:], func=AF.Abs,
                                     bias=NQ[:], scale=0.5)
                nc.scalar.activation(out=ad[:, 1:NR:2, :], in_=po[:], func=AF.Abs,
                                     bias=NQ[:], scale=0.5)
            # ---- x slab + column stencil images (rows r0-1..r0+NR) ----
            XS = xspool.tile([128, NR + 2, WP], BF16, tag="xs")
            for u in range(4):
                nc.sync.dma_start(out=XS[32 * u:32 * u + 32],
                                  in_=xq_in[u, :, r0 + 1:r0 + 1 + NR + 2, :])
            NH = NR + 2
            AR = abpool.tile([128, NH, WP], BF16, tag="ar")
            BR = abpool.tile([128, NH, WP], BF16, tag="br")
            B0t = abpool.tile([128, NH, WP], BF16, tag="br0")
            nc.vector.tensor_tensor(AR[:, :, 1:W + 1], XS[:, 1:NH + 1, 0:W],
                                    XS[:, 0:NH, 2:W + 2], OP.subtract)
            nc.gpsimd.tensor_tensor(B0t[:, :, 1:W + 1], XS[:, 1:NH + 1, 0:W],
                                    XS[:, 0:NH, 2:W + 2], OP.add)
            nc.vector.scalar_tensor_tensor(
                BR[:, :, 1:W + 1], in0=XS[:, 0:NH, 1:W + 1], scalar=-2.0,
                in1=B0t[:, :, 1:W + 1], op0=OP.mult, op1=OP.add)
            # reflect fixups at image cols 0 / W-1
            nc.vector.memset(AR[:, :, 1], 0.0)
            nc.vector.memset(AR[:, :, W], 0.0)
            nc.vector.tensor_tensor(BR[:, :, 1], BR[:, :, 1],
                                    XS[:, 1:NH + 1, 2], OP.add)
            nc.vector.tensor_tensor(BR[:, :, W], BR[:, :, W],
                                    XS[:, 1:NH + 1, W - 1], OP.add)
            # ---- H(dr) = X + DX*AR_dr + ADX*BR_dr on NR rows ----
            Hs = []
            for dr in (-1, 0, 1):
                a0 = 1 + dr
                x0 = 1 + dr
                T1 = hpool.tile([128, NR, W], BF16, tag="ht1", name=f"t1_{it}_{dr}")
                nc.vector.tensor_tensor(T1[:], DX[:], AR[:, a0:a0 + NR, 1:W + 1],
                                        OP.mult)
                T2 = hpool.tile([128, NR, W], BF16, tag="ht2", name=f"t2_{it}_{dr}")
                nc.vector.tensor_tensor(T2[:], ADX[:], BR[:, a0:a0 + NR, 1:W + 1],
                                        OP.mult)
                Hd = hpool.tile([128, NR, W], BF16, tag=f"h{dr}", name=f"h{dr}_{it}")
                nc.vector.tensor_tensor(Hd[:], T1[:], XS[:, x0:x0 + NR, 1:W + 1],
                                        OP.add)
                if dr == 0:
                    nc.gpsimd.tensor_tensor(Hd[:], Hd[:], T2[:], OP.add)
                else:
                    nc.vector.tensor_tensor(Hd[:], Hd[:], T2[:], OP.add)
                Hs.append(Hd)
            Hm, H0, Hp = Hs
            # ---- vertical combine ----
            BH = hpool.tile([128, NR, W], BF16, tag="bh", name=f"bh_{it}")
            nc.vector.tensor_tensor(BH[:], Hm[:], Hp[:], OP.add)
            AH = hpool.tile([128, NR, W], BF16, tag="ah", name=f"ah_{it}")
            nc.vector.tensor_tensor(AH[:], Hm[:], Hp[:], OP.subtract)
            H02 = hpool.tile([128, NR, W], BF16, tag="ht1", name=f"h02_{it}")
            nc.scalar.activation(out=H02[:], in_=H0[:], func=AF.Identity,
                                 bias=ZB[:], scale=2.0)
            nc.vector.tensor_tensor(BH[:], BH[:], H02[:], OP.subtract)
            if it == 0:
                nc.vector.memset(AH[0:32, 0, :], 0.0)
                nc.vector.tensor_tensor(BH[0:32, 0, :], BH[0:32, 0, :],
                                        Hp[0:32, 0, :], OP.add)
            if it == NSLAB - 1:
                nc.vector.memset(AH[96:128, NR - 1, :], 0.0)
                nc.vector.tensor_tensor(BH[96:128, NR - 1, :],
                                        BH[96:128, NR - 1, :],
                                        Hm[96:128, NR - 1, :], OP.add)
            T3 = hpool.tile([128, NR, W], BF16, tag="ht1", name=f"t3_{it}")
            nc.vector.tensor_tensor(T3[:], DY[:], AH[:], OP.mult)
            T4 = hpool.tile([128, NR, W], BF16, tag="ht2", name=f"t4_{it}")
            nc.vector.tensor_tensor(T4[:], ADY[:], BH[:], OP.mult)
            OSe = OSE[it % 3]
            OSo = OSO[it % 3]
            nc.vector.tensor_tensor(OSe[:, :, 1:W + 1], H0[:, 0:NR:2, :],
                                    T3[:, 0:NR:2, :], OP.add)
            nc.vector.tensor_tensor(OSo[:, :, 1:W + 1], H0[:, 1:NR:2, :],
                                    T3[:, 1:NR:2, :], OP.add)
            nc.vector.tensor_tensor(OSe[:, :, 1:W + 1], OSe[:, :, 1:W + 1],
                                    T4[:, 0:NR:2, :], OP.add)
            nc.vector.tensor_tensor(OSo[:, :, 1:W + 1], OSo[:, :, 1:W + 1],
                                    T4[:, 1:NR:2, :], OP.add)
            if it == 0:
                nc.vector.tensor_copy(STE[:, 0, :], OSe[:, 0, :])
                nc.vector.tensor_copy(STE[:, 1, :], OSe[:, 1, :])
                nc.vector.tensor_copy(STO[:, 0, :], OSo[:, 0, :])

        def dcn_group(u_or_none, ps, osr_rhs, t_pairs, skip):
            """3-tap col-paired matmuls + bias copy + stores.
            osr_rhs(v, kw) -> rhs AP; t_pairs[v] -> y row-pair index."""
            for v in range(2):
                for kw in range(3):
                    nc.tensor.matmul(
                        ps[64 * v:64 * v + 64, 0:W],
                        lhsT=WDCN[:, kw, 0:64],
                        rhs=osr_rhs(v, kw),
                        start=(kw == 0), stop=(kw == 2),
                        tile_position=(0, 64 * v),
                        skip_group_check=True)
            ST = stpool.tile([128, W], BF16, tag="st")
            nc.scalar.activation(out=ST[:], in_=ps[:, 0:W],
                                 func=AF.Identity, bias=BDCN[:], scale=1.0)
            for v in range(2):
                if skip[v]:
                    continue
                for h in range(2):
                    nc.sync.dma_start(
                        out=y_out[t_pairs[v], h],
                        in_=ST[64 * v + 32 * h:64 * v + 32 * h + 32, :])

        def dcn_slab(jt):
            R0 = jt * NR
            cur = jt % 3
            prv = (jt - 1) % 3
            nxt = (jt + 1) % 3
            OSR = osrpool.tile([128, 4, NPAIR, WP], BF16, tag="osr")
            for u in range(4):
                # g=0: odd rows R0-1, R0+1, R0+3, R0+5
                if jt == 0:
                    nc.vector.memset(OSR[0:32, u, 0, :], 0.0)
                else:
                    nc.sync.dma_start(out=OSR[0:32, u, 0, :],
                                      in_=OSO[prv][32 * u:32 * u + 32,
                                                   NPAIR - 1, :])
                nc.sync.dma_start(out=OSR[0:32, u, 1:NPAIR, :],
                                  in_=OSO[cur][32 * u:32 * u + 32, 0:NPAIR - 1, :])
                # g=1: even rows R0..R0+6
                nc.sync.dma_start(out=OSR[32:64, u], in_=OSE[cur][32 * u:32 * u + 32])
                # g=2: odd rows R0+1..R0+7
                nc.sync.dma_start(out=OSR[64:96, u], in_=OSO[cur][32 * u:32 * u + 32])
                # g=3: even rows R0+2..R0+8
                nc.sync.dma_start(out=OSR[96:128, u, 0:NPAIR - 1, :],
                                  in_=OSE[cur][32 * u:32 * u + 32, 1:NPAIR, :])
                if jt == NSLAB - 1:
                    nc.vector.memset(OSR[96:128, u, NPAIR - 1, :], 0.0)
                else:
                    nc.sync.dma_start(out=OSR[96:128, u, NPAIR - 1, :],
                                      in_=OSE[nxt][32 * u:32 * u + 32, 0, :])
            for u in range(4):
                t_base = (QH * u + R0) // 2
                for k in range(0, NPAIR, 2):
                    ps = opool.tile([128, 512], F32, tag="o")
                    skip = [False, False]
                    if jt == 0 and k == 0 and u >= 1:
                        skip[0] = True  # boundary pair t=48u -> cleanup
                    if jt == NSLAB - 1 and k == 2 and u <= 2:
                        skip[1] = True  # boundary pair t=48(u+1)-1 -> cleanup
                    dcn_group(
                        u, ps,
                        lambda v, kw, _u=u, _k=k: OSR[:, _u, _k + v, kw:kw + W],
                        [t_base + k, t_base + k + 1], skip)

        for it in range(NSLAB + 1):
            if it < NSLAB:
                sample_slab(it)
            if it >= 1:
                dcn_slab(it - 1)

        # ---- cleanup: quarter-boundary pairs t = 48v-1 and 48v, v=1..3 ----
        last = (NSLAB - 1) % 3
        for v in range(1, 4):
            O2 = osrpool.tile([128, 2, WP], BF16, tag="osr2", name=f"osr2_{v}")
            qm = 32 * (v - 1)
            qv = 32 * v
            # pair A = 48v-1: window rows 96v-3..96v
            nc.sync.dma_start(out=O2[0:32, 0, :],
                              in_=OSO[last][qm:qm + 32, NPAIR - 2, :])
            nc.sync.dma_start(out=O2[32:64, 0, :],
                              in_=OSE[last][qm:qm + 32, NPAIR - 1, :])
            nc.sync.dma_start(out=O2[64:96, 0, :],
                              in_=OSO[last][qm:qm + 32, NPAIR - 1, :])
            nc.sync.dma_start(out=O2[96:128, 0, :], in_=STE[qv:qv + 32, 0, :])
            # pair B = 48v: window rows 96v-1..96v+2
            nc.sync.dma_start(out=O2[0:32, 1, :],
                              in_=OSO[last][qm:qm + 32, NPAIR - 1, :])
            nc.sync.dma_start(out=O2[32:64, 1, :], in_=STE[qv:qv + 32, 0, :])
            nc.sync.dma_start(out=O2[64:96, 1, :], in_=STO[qv:qv + 32, 0, :])
            nc.sync.dma_start(out=O2[96:128, 1, :], in_=STE[qv:qv + 32, 1, :])
            ps = opool.tile([128, 512], F32, tag="o")
            dcn_group(
                None, ps,
                lambda vv, kw, _v=v: O2[:, vv, kw:kw + W],
                [48 * v - 1, 48 * v], [False, False])

    if finalize:
        nc.finalize()
    return nc


def prep_x(x_img):
    """Host-side packing for one core. x_img: [C, H, W] f32."""
    xb = np.asarray(x_img).astype(ml_dtypes.bfloat16)
    xpad = np.zeros((C, H + 2, WP), dtype=ml_dtypes.bfloat16)
    xpad[:, 1:H + 1, 1:W + 1] = xb
    xrs = np.ascontiguousarray(
        np.stack([xpad[:, g:g + H:2, :] for g in range(4)], axis=0))
    xpad2 = np.zeros((C, H + 4, WP), dtype=ml_dtypes.bfloat16)
    xpad2[:, 2:H + 2, 1:W + 1] = xb
    xqs = np.ascontiguousarray(
        np.stack([xpad2[:, QH * u:QH * u + QH + 4, :] for u in range(4)], axis=0))
    return xrs, xqs


def prep_weights(w_off, b_off, w_dcn, b_dcn):
    woff = np.zeros((128, 3, 128), dtype=np.float32)
    wdcn = np.zeros((128, 3, 64), dtype=np.float32)
    for g in range(4):
        for h in range(2):
            kh = g - h
            if 0 <= kh <= 2:
                for axis in range(2):
                    woff[32 * g:32 * g + 32, :,
                         64 * h + 32 * axis:64 * h + 32 * axis + 32] = \
                        w_off[axis::2, :, kh, :].transpose(1, 2, 0)
                wdcn[32 * g:32 * g + 32, :, 32 * h:32 * h + 32] = \
                    w_dcn[:, :, kh, :].transpose(1, 2, 0)
    boff = np.zeros((128, 1), np.float32)
    for h in range(2):
        for axis in range(2):
            boff[64 * h + 32 * axis:64 * h + 32 * axis + 32, 0] = b_off[axis::2]
    bdcn = np.zeros((128, 1), np.float32)
    for v in range(2):
        for h in range(2):
            bdcn[64 * v + 32 * h:64 * v + 32 * h + 32, 0] = b_dcn
    return {
        "woff": np.ascontiguousarray(
            woff.reshape(128, 3 * 128)).astype(ml_dtypes.bfloat16),
        "wdcn": np.ascontiguousarray(
            wdcn.reshape(128, 3 * 64)).astype(ml_dtypes.bfloat16),
        "boff": boff, "bdcn": bdcn,
    }


_NC_CACHE = {}


def _get_nc():
    if "nc" not in _NC_CACHE:
        _NC_CACHE["nc"] = build_nc()
    return _NC_CACHE["nc"]


def _run(x, w_off, b_off, w_dcn, b_dcn, **spmd_kwargs):
    from concourse.bass_utils import run_bass_kernel_spmd

    B = x.shape[0]
    assert x.shape == (B, C, H, W) and B == N_CORES
    nc = _get_nc()
    w = prep_weights(np.asarray(w_off, dtype=np.float32),
                     np.asarray(b_off, dtype=np.float32),
                     np.asarray(w_dcn, dtype=np.float32),
                     np.asarray(b_dcn, dtype=np.float32))
    in_maps = []
    xnp = np.asarray(x)
    for b in range(B):
        m = dict(w)
        m["xr"], m["xq"] = prep_x(xnp[b])
        in_maps.append(m)
    return run_bass_kernel_spmd(nc, in_maps, list(range(N_CORES)), **spmd_kwargs)


def kernel(x, w_off, b_off, w_dcn, b_dcn):
    res = _run(x, w_off, b_off, w_dcn, b_dcn)
    outs = []
    for i in range(N_CORES):
        y = np.asarray(res.results[i]["y"]).astype(np.float32)  # [192,2,32,384]
        outs.append(y.reshape(H, C, W).transpose(1, 0, 2))
    return np.stack(outs, axis=0)
